# revision 27
# baseline (speedup 1.0000x reference)
"""AttentionMixer kernel for 8 Trainium2 NeuronCores.

Sharding: data-parallel over (batch B=4) x (query-half NQ/2) -> 8 cores.
Each core computes, for its (b, half):
    q = meshT slice proj, k/v = pc proj (k/v work duplicated across the
    2 cores of a batch), masked softmax attention, Wo projection.
Layout is "transposed" throughout (features on partitions, tokens on the
free dim) so every matmul contracts over the partition dim natively:
    qT/kT: [e, n] via W.T as lhsT, xT as rhs
    scoresT: [nk, nq] = kT_h.T-contract-d qT_h  (2 heads row-packed)
    attnT = exp(scoresT/8 + mask_bias)          (one ACT op per tile)
    ctxT_h: [65, nq] via v_aug lhsT (ones column -> softmax denom Z for
    free), normalized post-hoc: mix = (attn@v)@Wo.T / Z + (Wo@bv + bo).
All big matmuls bf16 with fp32 PSUM accumulation.

The j-loop (nk blocks) is software-pipelined: scores/exp of step j are
emitted before ctx of step j-1 so PE never stalls behind the current
exp.  The kernel is compiled for jmax = ceil(max(lengths)/128) nk
blocks — key positions beyond a batch's length are masked to exp(-80)
~= 0, so blocks beyond jmax contribute nothing and are skipped
uniformly across cores (SPMD-preserving).
"""

import math

import numpy as np
import ml_dtypes

import concourse.bass as bass
import concourse.bacc as bacc
import concourse.mybir as mybir
import concourse.tile as tile
from concourse.bass_utils import run_bass_kernel_spmd

B, NQ, NK, E, DPC, H = 4, 2048, 4096, 256, 128, 4
HD = E // H  # 64
NQH = NQ // 2  # per-core queries: 1024
NKB = NK // 128  # 32 nk blocks
P = 128
BF16 = mybir.dt.bfloat16
F32 = mybir.dt.float32
MASK_NEG = -80.0

_CACHE = {}


def build_nc(jmax=NKB):
    nc = bacc.Bacc(None)
    knt = (jmax + 3) // 4        # 512-wide kT tiles needed
    nch = (jmax + 7) // 8        # 1024-wide pcT DMA chunks needed

    # ---- DRAM params (per-core shapes; host stages exact SBUF layouts) ----
    meshT_d = nc.declare_dram_parameter("meshT", [P, 2, NQH], BF16, False)
    pcT_d = nc.declare_dram_parameter("pcT", [P, NK], BF16, False)
    wqT_d = nc.declare_dram_parameter("wqT", [P, 2, E], BF16, False)
    wkT_d = nc.declare_dram_parameter("wkT", [P, E], BF16, False)
    wvT_d = nc.declare_dram_parameter("wvT", [P, E], BF16, False)
    woT_d = nc.declare_dram_parameter("woT", [HD, H, E], BF16, False)
    # consts: [bk | bq | bop | maskb] along the free dim
    consts_d = nc.declare_dram_parameter("consts", [P, 6 + NKB], F32, False)
    mixT_d = nc.declare_dram_parameter("mixT", [2, P, NQH], F32, isOutput=True)

    with tile.TileContext(nc) as tc:
        with (
            tc.tile_pool(name="const", bufs=1) as cpool,
            tc.tile_pool(name="acts", bufs=1) as apool,
            tc.tile_pool(name="attn", bufs=4) as attn_pool,
            tc.tile_pool(name="small", bufs=2) as spool,
            tc.tile_pool(name="ps_big", bufs=2, space="PSUM") as ps_big,
            tc.tile_pool(name="ps_ctx", bufs=4, space="PSUM") as ps_ctx,
        ):
            # ---- load constants / inputs into SBUF ----
            meshT = cpool.tile([P, 2, NQH], BF16)
            pcT = cpool.tile([P, NK], BF16)
            wqT = cpool.tile([P, 2, E], BF16)
            wkT = cpool.tile([P, E], BF16)
            wvT = cpool.tile([P, E], BF16)
            woT = cpool.tile([HD, H, E], BF16)
            consts = cpool.tile([P, 6 + NKB], F32)
            bk = consts[:, 0:2]
            bq = consts[:, 2:4]
            bop = consts[:, 4:6]
            maskb = consts[:, 6:6 + NKB]

            # critical-path inputs on the sync engine (HWDGE); the rest
            # ride gpsimd's SWDGE so they don't queue ahead of meshT/wqT
            nc.sync.dma_start(wkT[:], wkT_d[:, :])
            nc.sync.dma_start(consts[:], consts_d[:, :])
            nc.sync.dma_start(pcT[:, 0:512], pcT_d[:, 0:512])
            nc.sync.dma_start(wqT[:], wqT_d[:, :, :])
            nc.sync.dma_start(meshT[:], meshT_d[:, :, :])
            nc.sync.dma_start(pcT[:, 512:1024], pcT_d[:, 512:1024])
            for ch in range(1, nch):
                nc.sync.dma_start(pcT[:, ch * 1024:(ch + 1) * 1024],
                                  pcT_d[:, ch * 1024:(ch + 1) * 1024])
            nc.sync.dma_start(wvT[:], wvT_d[:, :])
            nc.sync.dma_start(woT[:], woT_d[:, :, :])

            kT = apool.tile([P, 2, NK], BF16)
            qT = apool.tile([P, 2, NQH], BF16)
            v_sb = apool.tile([P, NKB, H * (HD + 1)], BF16)
            for h in range(H):
                nc.vector.memset(v_sb[:, :, h * 65 + 64:h * 65 + 65], 1.0)
            mixT = apool.tile([P, 2, NQH], F32)
            ctxn = apool.tile([HD, H, NQH], BF16)  # normalized ctxT per head

            def k_proj_eb(eb, nt0, n_nt):
                # n_nt (1 or 2) 512-wide kT tiles for one e-block
                ps = ps_big.tile([P, 1024], F32, tag="big")
                for i in range(n_nt):
                    nc.tensor.matmul(
                        ps[:, i * 512:(i + 1) * 512],
                        wkT[:, eb * P:(eb + 1) * P],
                        pcT[:, (nt0 + i) * 512:(nt0 + i + 1) * 512],
                        start=True, stop=True,
                    )
                nc.vector.tensor_scalar_add(
                    kT[:, eb, nt0 * 512:(nt0 + n_nt) * 512],
                    ps[:, 0:n_nt * 512], bk[:, eb:eb + 1])

            def q_proj_ebnt(eb, nt):
                ps = ps_big.tile([P, 1024], F32, tag="big")
                for cb in range(2):
                    nc.tensor.matmul(
                        ps[:, 0:512],
                        wqT[:, cb, eb * P:(eb + 1) * P],
                        meshT[:, cb, nt * 512:(nt + 1) * 512],
                        start=(cb == 0), stop=(cb == 1),
                    )
                nc.vector.tensor_scalar_add(
                    qT[:, eb, nt * 512:(nt + 1) * 512], ps[:, 0:512],
                    bq[:, eb:eb + 1])

            def v_proj(j):
                ps = ps_ctx.tile([P, E], F32, tag="ctx")
                nc.tensor.matmul(
                    ps[:],
                    pcT[:, j * P:(j + 1) * P],
                    wvT[:],
                    start=True, stop=True,
                )
                vdst = v_sb[:, j, :].rearrange("p (h x) -> p h x", x=HD + 1)
                nc.vector.tensor_copy(
                    vdst[:, :, 0:HD],
                    ps[:].rearrange("p (h x) -> p h x", x=HD))

            def wo_proj(nt, ebs=(0, 1)):
                # mixT[e'] = sum_h WoT_h.T @ ctxn_h  (+ bop, on DVE)
                for eb in ebs:
                    ps = ps_big.tile([P, 1024], F32, tag="big")
                    for h in range(H):
                        nc.tensor.matmul(
                            ps[:, 0:512],
                            woT[:, h, eb * P:(eb + 1) * P],
                            ctxn[:, h, nt * 512:(nt + 1) * 512],
                            start=(h == 0), stop=(h == H - 1),
                        )
                    nc.vector.tensor_scalar_add(
                        mixT[:, eb, nt * 512:(nt + 1) * 512], ps[:, 0:512],
                        bop[:, eb:eb + 1])
                    nc.sync.dma_start(
                        mixT_d[eb][:, nt * 512:(nt + 1) * 512],
                        mixT[:, eb, nt * 512:(nt + 1) * 512])

            # k tiles 0-1 (pcT chunk 0), then q, then the remaining k
            # tiles (these fill the PE's DMA-wait window); v is
            # interleaved into the first attention pass
            # pre-loop: only what the first scores/ctx steps need; the
            # rest of the projections trickle into the pass-0 j-loop so
            # the first exp starts as early as possible
            k_proj_eb(0, 0, 2)
            q_proj_ebnt(0, 0)
            for j in range(2):
                v_proj(j)    # prologue: v a couple blocks ahead of ctx
            extras = []
            ke0 = [("k", 0, nt0, min(2, knt - nt0))
                   for nt0 in range(2, knt, 2)]
            ke1 = [("k", 1, nt0, min(2, knt - nt0))
                   for nt0 in range(0, knt, 2)]
            if ke0:
                extras.append(ke0.pop(0))
            extras.append(ke1.pop(0))
            extras.extend(ke0)
            extras.append(("q", 1, 0, 0))
            extras.extend(ke1)
            extras.append(("q", 0, 1, 0))
            extras.append(("q", 1, 1, 0))

            # ---- attention main loop (software-pipelined) ----
            VLEAD = 4
            # nt=0 passes first so the nt=0 output projection overlaps the
            # nt=1 passes
            passes = [(0, 0), (1, 0), (0, 1), (1, 1)]
            for pi, (hp, nt) in enumerate(passes):
                h0, h1 = 2 * hp, 2 * hp + 1
                acc0 = ps_ctx.tile([HD + 1, 512], F32, tag="ctx")
                acc1 = ps_ctx.tile([HD + 1, 512], F32, tag="ctx")
                pend = None
                for j in range(jmax):
                    s = ps_big.tile([P, 1024], F32, tag="big")
                    # scores for the two heads -> adjacent psum banks;
                    # the 64-row lhsT slices land on disjoint row groups
                    # so the pair runs concurrently on the PE
                    nc.tensor.matmul(
                        s[:, 0:512],
                        kT[0:HD, hp, j * P:(j + 1) * P],
                        qT[0:HD, hp, nt * 512:(nt + 1) * 512],
                        start=True, stop=True,
                    )
                    nc.tensor.matmul(
                        s[:, 512:1024],
                        kT[HD:P, hp, j * P:(j + 1) * P],
                        qT[HD:P, hp, nt * 512:(nt + 1) * 512],
                        start=True, stop=True,
                    )
                    a = attn_pool.tile([P, 1024], BF16, tag="attn")
                    nc.scalar.activation(
                        a[:], s[:],
                        mybir.ActivationFunctionType.Exp,
                        bias=maskb[:, j:j + 1], scale=0.125)
                    if pi == 0 and j + 2 < jmax:
                        v_proj(j + 2)
                    if pi == 0 and j % 2 == 1 and (j - 1) // 2 < len(extras):
                        kind, eb, nt0, n_nt = extras[(j - 1) // 2]
                        if kind == "k":
                            k_proj_eb(eb, nt0, n_nt)
                        else:
                            q_proj_ebnt(eb, nt0)
                    if pi == 2 and j == 6:
                        wo_proj(0, ebs=(0,))
                    if pi == 2 and j == 12:
                        wo_proj(0, ebs=(1,))
                    if pend is not None:
                        ap, jp = pend
                        nc.tensor.matmul(
                            acc0[:],
                            v_sb[:, jp, h0 * 65:(h0 + 1) * 65],
                            ap[:, 0:512],
                            start=(jp == 0), stop=False,
                        )
                        nc.tensor.matmul(
                            acc1[:],
                            v_sb[:, jp, h1 * 65:(h1 + 1) * 65],
                            ap[:, 512:1024],
                            start=(jp == 0), stop=False,
                        )
                    pend = (a, j)
                ap, jp = pend
                nc.tensor.matmul(
                    acc0[:], v_sb[:, jp, h0 * 65:(h0 + 1) * 65],
                    ap[:, 0:512], start=(jp == 0), stop=True)
                nc.tensor.matmul(
                    acc1[:], v_sb[:, jp, h1 * 65:(h1 + 1) * 65],
                    ap[:, 512:1024], start=(jp == 0), stop=True)
                # normalize: ctx[0:64] / Z (Z = row 64)
                for h, acc in ((h0, acc0), (h1, acc1)):
                    zrow = spool.tile([1, 512], F32, tag="zrow")
                    nc.vector.tensor_copy(zrow[:], acc[HD:HD + 1, :])
                    zr = spool.tile([1, 512], F32, tag="zr")
                    nc.vector.reciprocal_approx_fast(zr[:], zrow[:])
                    # broadcast 1/Z to 64 partitions (gpsimd; no PSUM)
                    zbs = spool.tile([HD, 512], F32, tag="zbs")
                    nc.gpsimd.partition_broadcast(zbs[:], zr[:])
                    nc.vector.tensor_mul(
                        ctxn[:, h, nt * 512:(nt + 1) * 512],
                        acc[0:HD, :], zbs[:])
            wo_proj(1)

    nc.finalize()
    return nc


def _get_nc(jmax):
    if jmax not in _CACHE:
        _CACHE[jmax] = build_nc(jmax)
    return _CACHE[jmax]


def kernel(mesh_feats, pc_feats, Wq, Wk, Wv, bq, bk, bv, Wo, bo, lengths,
           _trace=False, _trace_kwargs=None):
    mesh_feats = np.asarray(mesh_feats, np.float32)
    pc_feats = np.asarray(pc_feats, np.float32)
    Wq, Wk, Wv = (np.asarray(x, np.float32) for x in (Wq, Wk, Wv))
    bqv, bkv, bvv = (np.asarray(x, np.float32) for x in (bq, bk, bv))
    Wo, bo = np.asarray(Wo, np.float32), np.asarray(bo, np.float32)
    lengths = np.asarray(lengths, np.int32)

    bf = ml_dtypes.bfloat16
    wqT = np.ascontiguousarray(
        Wq.T.reshape(2, P, E).transpose(1, 0, 2)).astype(bf)   # [128, 2, 256]
    wkT = np.ascontiguousarray(Wk.T).astype(bf)          # [128, 256]
    wvT = np.ascontiguousarray(Wv.T).astype(bf)          # [128, 256]
    woT = np.ascontiguousarray(
        Wo.T.reshape(H, HD, E).transpose(1, 0, 2)).astype(bf)  # [64, 4, 256]
    bq2 = np.ascontiguousarray(bqv.reshape(2, P).T)      # [128, 2]
    bk2 = np.ascontiguousarray(bkv.reshape(2, P).T)
    bop = Wo @ bvv + bo
    bop2 = np.ascontiguousarray(bop.reshape(2, P).T)

    jmax = int(min(NKB, max(1, math.ceil(int(lengths.max()) / 128))))

    idx = np.arange(NK).reshape(NKB, P).T                # [128, 32]
    in_maps = []
    for c in range(8):
        b, half = c // 2, c % 2
        meshT = np.ascontiguousarray(
            mesh_feats[b, half * NQH:(half + 1) * NQH, :].T
            .reshape(2, P, NQH).transpose(1, 0, 2)).astype(bf)  # [128,2,1024]
        pcT = np.ascontiguousarray(pc_feats[b].T).astype(bf)
        maskb = np.where(idx < int(lengths[b]), 0.0, MASK_NEG).astype(np.float32)
        consts = np.ascontiguousarray(
            np.concatenate([bk2, bq2, bop2, maskb], axis=1).astype(np.float32))
        in_maps.append({
            "meshT": meshT, "pcT": pcT, "wqT": wqT, "wkT": wkT,
            "wvT": wvT, "woT": woT, "consts": consts,
        })

    nc = _get_nc(jmax)
    res = run_bass_kernel_spmd(
        nc, in_maps, list(range(8)),
        trace=_trace, **(_trace_kwargs or {}))
    out = np.empty((B, NQ, 2 * E), np.float32)
    out[:, :, :E] = mesh_feats
    for c in range(8):
        b, half = c // 2, c % 2
        mixT = res.results[c]["mixT"]            # [2, 128, NQH]
        out[b, half * NQH:(half + 1) * NQH, E:] = mixT.reshape(E, NQH).T
    if _trace:
        return out, res
    return out
